# revision 60
# baseline (speedup 1.0000x reference)
"""Trainium2 Bass kernel for nn_Attention_30227979829300.

Multi-head attention (b=4, n=2048, dim=1024, 16 heads x 64) with
interleaved-pair RoPE + Fourier positional encoding, sharded across 8
NeuronCores as batch x head-group: core c handles batch c%4 and heads
[8*(c//4), 8*(c//4)+8) (4 head-pairs).  Per-core attention work is the
same as pure head-sharding (8 head-pair units), but tokens-per-core
drops 4x, so the out-projection partial, its PSUM->SBUF staging (the
scarce ACT/DVE psum-port resource) and the input/output HBM traffic all
shrink 4x.

Other key structure:
  - scores for a head-pair are emitted as row-tiled matmul pairs
    (tile_position (0,0)/(64,0)): K=64 each, both run concurrently in
    disjoint row-groups (HW-verified pair span 216ns = one matmul).
  - softmax exp is split across engines: ACT does bf16 exp for head 0;
    head 1 goes to the DVE as a Schraudolph int16 trick
    (round(x*128/ln2 + B) bitcast as bf16 ~= e^x to +-3.3%), so the two
    sg PSUM tiles of a jc free near-simultaneously and the next score
    pair issues concurrently.
  - the jc loop windows keep the engine queues clear for exp: RoPE
    combine, v transposes and og staging are deferred to the post-loop
    attn@V replay stretches (pure PE work, no exp dependency).
  - out partials written bf16 on the GpSimd SWDGE queue.
Host sums partials per batch (the head-group all-reduce) and adds b_out.

Per-core layouts (transposed so softmax needs no on-chip transposes and
no max-subtraction):
  - qkv projection: q^T/k^T/v^T in [head_dim, tokens] bf16, PSUM f32
  - rot(q)/rot(k) via block-diag signed permutation matmul
  - RoPE: q_rope = q*cos + rot(q)*sin + fenc
  - scores s^T[j, i] = sum_d k[j,d] q[i,d]  (j on partitions)
  - p = exp(s/8) straight out of PSUM; attn@V with a fused ones column
    in the stationary operand producing the denominator row for free
  - normalize via fast-reciprocal + GpSimd partition broadcast
  - out-projection accumulates the 4 head-pairs into token-major
    [tokens, 1024] partials
"""

import sys

if "/opt/trn_rl_repo" not in sys.path:
    sys.path.insert(0, "/opt/trn_rl_repo")

import numpy as np
import ml_dtypes

import concourse.bass as bass
import concourse.tile as tile
from concourse import bacc, mybir
from concourse.bass_utils import run_bass_kernel_spmd

F32 = mybir.dt.float32
BF16 = mybir.dt.bfloat16
I16 = mybir.dt.int16
ACT_EXP = mybir.ActivationFunctionType.Exp
NP_BF16 = ml_dtypes.bfloat16

B, N, DIM = 4, 2048, 1024
HEADS, DH = 16, 64
INNER = HEADS * DH
NF = 16  # fourier freqs
NCORES = 8
NHP = 4  # head-pairs per core
SCALE = DH ** -0.5

# Schraudolph bf16-bitcast exp: int16 y = round(x * 128/ln2 + B16);
# bitcast(y) ~= e^x with max rel err ~3.3% (B16 centered on HW-measured
# log-ratio range).
LN2 = float(np.log(2.0))
A16 = 128.0 / LN2
B16 = 127 * 128 - 5.51

# per-jc exp engine for (head0, head1); "dve" = Schraudolph trick.
EXP_SCHED = [("act", "dve") for jc in range(16)]


def _build_program():
    nc = bacc.Bacc("TRN2", target_bir_lowering=False, debug=False,
                   num_devices=NCORES)

    d = lambda name, shape, dt, kind: nc.dram_tensor(name, shape, dt, kind=kind).ap()
    xT = d("xT", [DIM, N], BF16, "ExternalInput")
    wq = d("wq", [DIM, NHP * 128], BF16, "ExternalInput")
    wk = d("wk", [DIM, NHP * 128], BF16, "ExternalInput")
    wv = d("wv", [DIM, NHP * 128], BF16, "ExternalInput")
    rotmT = d("rotmT", [128, 128], BF16, "ExternalInput")
    wo = d("wo", [NHP * 128, DIM], BF16, "ExternalInput")
    cos2 = d("cos2", [128, N], BF16, "ExternalInput")
    sin2 = d("sin2", [128, N], BF16, "ExternalInput")
    fourT = d("fourT", [2 * NF, N], BF16, "ExternalInput")
    wfT = d("wfT", [2 * NF, DH], BF16, "ExternalInput")
    bf = d("bf", [DH, 1], F32, "ExternalInput")
    ident = d("ident", [128, 128], BF16, "ExternalInput")
    onesv = d("onesv", [128, 32], BF16, "ExternalInput")
    out = d("out", [N, DIM], BF16, "ExternalOutput")

    with tile.TileContext(nc) as tc:
        with tc.tile_pool(name="consts", bufs=1) as consts, \
             tc.tile_pool(name="xt", bufs=12) as xtp, \
             tc.tile_pool(name="qk", bufs=4) as qkp, \
             tc.tile_pool(name="vsb", bufs=4) as vsbp, \
             tc.tile_pool(name="sbc", bufs=2) as sbcp, \
             tc.tile_pool(name="vtmp", bufs=2) as vtmpp, \
             tc.tile_pool(name="ptil", bufs=32) as ptilp, \
             tc.tile_pool(name="ropet", bufs=2) as ropetp, \
             tc.tile_pool(name="outT", bufs=16) as outTp, \
             tc.tile_pool(name="ostg", bufs=2) as ostgp, \
             tc.tile_pool(name="arow", bufs=1) as arowp, \
             tc.tile_pool(name="acc", bufs=2, space="PSUM") as accp, \
             tc.tile_pool(name="pacc", bufs=1, space="PSUM") as paccp, \
             tc.tile_pool(name="avacc", bufs=2, space="PSUM") as avaccp, \
             tc.tile_pool(name="small", bufs=1, space="PSUM") as smallp:

            # ---- load constants ----
            # DMA issue order is latency-critical at startup: tensors needed
            # by the first PE ops go first.
            four_sb = consts.tile([2 * NF, N], BF16, tag="four")
            nc.sync.dma_start(four_sb[:], fourT[:])
            wf_sb = consts.tile([2 * NF, DH], BF16, tag="wf")
            nc.sync.dma_start(wf_sb[:], wfT[:])
            bf_sb = consts.tile([DH, 1], F32, tag="bf")
            nc.sync.dma_start(bf_sb[:], bf[:])

            w_sb = {}
            for name, ap in (("wq", wq), ("wk", wk), ("wv", wv)):
                w_sb[name] = consts.tile([128, 8 * NHP * 128], BF16,
                                         tag=name, name=f"wsb_{name}")

            def load_w_chunk(name, ap, fc):
                # one 128-row chunk: [128, NHP*128] cols
                t = w_sb[name]
                nc.sync.dma_start(
                    t[:, fc * NHP * 128:(fc + 1) * NHP * 128],
                    ap[fc * 128:(fc + 1) * 128, :])

            # x blocks are re-fetched per head-pair (keeping all of x
            # resident does not fit SBUF next to 4 head-pairs' q/k/v)
            xts_pre = {}

            def prefetch_xt(hp, blk):
                if (hp, blk) in xts_pre or hp >= NHP:
                    return
                xts = []
                for fc in range(8):
                    xt_t = xtp.tile([128, 512], BF16, tag="xt")
                    nc.sync.dma_start(
                        xt_t[:],
                        xT[bass.ts(fc, 128), blk * 512:(blk + 1) * 512])
                    xts.append(xt_t)
                xts_pre[(hp, blk)] = xts

            # interleave per-chunk weight DMAs with block 0's x tiles so the
            # first projection matmuls start ~1.5us in, not after 3MB.
            # (Block 1's tiles must come after ALL of block 0's: the 12-slot
            # ring reuses block-0 slots, and an interleaved order creates a
            # PE-FIFO cycle through the in-order DMA queue.)
            xts_pre[(0, 0)] = []
            for fc in range(8):
                load_w_chunk("wq", wq, fc)
                xt_t = xtp.tile([128, 512], BF16, tag="xt")
                nc.sync.dma_start(xt_t[:], xT[bass.ts(fc, 128), 0:512])
                xts_pre[(0, 0)].append(xt_t)
            for fc in range(8):
                load_w_chunk("wk", wk, fc)
            prefetch_xt(0, 1)
            for fc in range(8):
                load_w_chunk("wv", wv, fc)
            sin_sb = consts.tile([128, N], BF16, tag="sin")
            nc.sync.dma_start(sin_sb[:], sin2[:])
            rotm_sb = consts.tile([128, 128], BF16, tag="rotm")
            nc.sync.dma_start(rotm_sb[:], rotmT[:])
            onesv_sb = consts.tile([128, 32], BF16, tag="onesv")
            nc.sync.dma_start(onesv_sb[:], onesv[:])
            cos_sb = consts.tile([128, N], BF16, tag="cos")
            nc.sync.dma_start(cos_sb[:], cos2[:])
            id_sb = consts.tile([128, 128], BF16, tag="ident")
            nc.sync.dma_start(id_sb[:], ident[:])
            wo_sb = consts.tile([128, NHP * DIM], BF16, tag="wo")
            nc.sync.dma_start(
                wo_sb[:].rearrange("p (h d) -> p h d", h=NHP),
                wo.rearrange("(h p) d -> p h d", p=128))

            # ---- fenc2 [128, 2048]: fourier @ w_fproj.T + b_fproj ----
            fenc_sb = consts.tile([128, N], BF16, tag="fenc")
            for blk in range(4):
                fp = smallp.tile([DH, 512], F32, tag="small")
                nc.tensor.matmul(fp[:], wf_sb[:], four_sb[:, bass.ts(blk, 512)],
                                 start=True, stop=True)
                nc.scalar.add(fenc_sb[0:64, bass.ts(blk, 512)], fp[:], bf_sb[:])
                nc.scalar.add(fenc_sb[64:128, bass.ts(blk, 512)], fp[:], bf_sb[:])

            hp_tiles = {}

            def proj_block_gen(hp, blk, defer_list=None):
                """Project q/k/v for 512 tokens for head-pair hp, RoPE (with
                on-chip rotate_half permutation matmuls), transpose v to
                natural layout.  Yields between small chunks of PE work.
                Only the matmuls and the PSUM-freeing CAST/t2 ops run inline
                (keeping the exp queues clear); the rope combine and v
                transposes go onto defer_list for the post-loop stretches."""
                if blk == 0:
                    q_rope = qkp.tile([128, N], BF16, tag="q")
                    k_rope = qkp.tile([128, N], BF16, tag="k")
                    v_sb = vsbp.tile([128, 16 * 130], BF16, tag="v")
                    # ones columns (col 64 of each [65]-block, both heads)
                    nc.vector.tensor_copy(
                        bass.AP(tensor=v_sb[:].tensor, offset=v_sb[:].offset + 64,
                                ap=[v_sb[:].ap[0], [130, 16], [65, 2]]),
                        onesv_sb[:].rearrange("p (a t) -> p a t", t=2))
                    hp_tiles[hp] = (q_rope, k_rope, v_sb)
                q_rope, k_rope, v_sb = hp_tiles[hp]
                prefetch_xt(hp, blk)
                xts = xts_pre.pop((hp, blk))
                yield
                sbt = {}
                for u, stag in (("wq", "qsb"), ("wk", "ksb"), ("wv", "vt")):
                    p = paccp.tile([128, 512], F32, tag="pacc")
                    for fc in range(8):
                        nc.tensor.matmul(
                            p[:],
                            w_sb[u][:, fc * NHP * 128 + hp * 128:
                                    fc * NHP * 128 + hp * 128 + 128],
                            xts[fc][:],
                            start=(fc == 0), stop=(fc == 7))
                        if fc % 2 == 1:
                            yield
                    # PSUM -> SBUF bf16 staging (frees the pacc slot)
                    pool = vtmpp if u == "wv" else sbcp
                    s = pool.tile([128, 512], BF16, tag=stag)
                    nc.vector.tensor_copy(s[:], p[:])
                    sbt[u] = s
                    yield
                q_sb, k_sb, vt = sbt["wq"], sbt["wk"], sbt["wv"]
                # rotate_half via signed permutation matmul into the small
                # PSUM bank; t2 runs inline to free it immediately
                bsl = bass.ts(blk, 512)
                ropes = []
                for src, dst in ((q_sb, q_rope), (k_sb, k_rope)):
                    rr = smallp.tile([128, 512], F32, tag="small")
                    nc.tensor.matmul(rr[:], rotm_sb[:], src[:], start=True,
                                     stop=True)
                    t2 = ropetp.tile([128, 512], BF16, tag="t2")
                    nc.vector.tensor_mul(t2[:], rr[:], sin_sb[:, bsl])
                    ropes.append((src, t2, dst))
                    yield

                def deferred():
                    for src, t2, dst in ropes:
                        t1 = ropetp.tile([128, 512], BF16, tag="t1", bufs=1)
                        nc.vector.tensor_mul(t1[:], src[:], cos_sb[:, bsl])
                        t3 = ropetp.tile([128, 512], BF16, tag="t3", bufs=1)
                        nc.vector.tensor_add(t3[:], t1[:], t2[:])
                        nc.vector.tensor_add(dst[:, bsl], t3[:],
                                             fenc_sb[:, bsl])
                        yield
                    for tt in range(4):
                        jc = blk * 4 + tt
                        ptp = smallp.tile([128, 128], BF16, tag="small")
                        nc.tensor.transpose(ptp[:], vt[:, bass.ts(tt, 128)],
                                            id_sb[:])
                        # both 64-col head halves in one strided copy,
                        # skipping the ones column at +64
                        nc.vector.tensor_copy(
                            bass.AP(tensor=v_sb[:].tensor,
                                    offset=v_sb[:].offset + jc * 130,
                                    ap=[v_sb[:].ap[0], [65, 2], [1, 64]]),
                            ptp[:].rearrange("p (a t) -> p a t", a=2))
                        yield

                if defer_list is None:
                    for _ in deferred():
                        pass
                else:
                    defer_list.append(deferred())

            ots = {}  # (hp, half-of-pb) -> ot tile

            def outproj_gen(ib):
                """Out-projection for token block ib (512 tokens),
                contracting all 4 head-pairs into one [512, 1024] partial."""
                pb, half = ib // 2, ib % 2
                for ic in range(4):
                    for oc in range(2):
                        po = smallp.tile([128, 512], F32, tag="small")
                        for hp in range(NHP):
                            nc.tensor.matmul(
                                po[:], ots[(hp, pb, half)][:, bass.ts(ic, 128)],
                                wo_sb[:, hp * DIM + oc * 512:
                                      hp * DIM + oc * 512 + 512],
                                start=(hp == 0), stop=(hp == NHP - 1))
                            if hp == 1:
                                yield
                        og = ostgp.tile([128, 512], BF16, tag="og")
                        # alternate the PSUM drain between ACT and DVE
                        if (ic * 2 + oc) % 2 == 0:
                            nc.scalar.copy(og[:], po[:])
                        else:
                            nc.vector.tensor_copy(og[:], po[:])
                        r0 = ib * 512 + ic * 128
                        # bf16 partial on the SWDGE (GpSimd) queue: keeps the
                        # Sync HWDGE ring free for input prefetch
                        nc.gpsimd.dma_start(
                            out[r0:r0 + 128, bass.ts(oc, 512)], og[:])
                        yield

            def attn_tail(op_, ot, hp_half):
                # denominator row staged via ACT (reciprocal_approx_fast is a
                # custom DVE op that cannot read PSUM), then reciprocal +
                # GpSimd partition broadcast + normalize (all base-0 tiles:
                # the custom ops require partition offset 0)
                ar0 = arowp.tile([1, 512], F32, tag="ar0")
                nc.scalar.copy(ar0[:], op_[64:65, :])
                ar = arowp.tile([1, 512], F32, tag="ar")
                nc.vector.reciprocal_approx_fast(ar[:], ar0[:])
                bc = arowp.tile([64, 512], F32, tag="bc")
                nc.gpsimd.partition_broadcast(bc[:], ar[:])
                nc.vector.tensor_mul(ot[hp_half, :], op_[0:64, :], bc[:])

            def attn_unit(hp, pb, ot0, ot1, drive, drive_post,
                          tail_hook=None, gate=None, cad=2):
                """Both heads of head-pair hp over a 1024-token i-pair:
                scores (row-tiled concurrent pairs) + exp + attn@V.  The jc
                loop computes h0's first-half attn@V inline as exp lands;
                h1's first half and both second halves replay the 32
                resident pt tiles in exp-free stretches, which also absorb
                the deferred rope/og work."""
                q_rope, k_rope, v_sb = hp_tiles[hp]
                isl = lambda ih: slice(pb * 1024 + ih * 512,
                                       pb * 1024 + (ih + 1) * 512)
                pts = {}

                def emit_av(op_, h, jc, half):
                    nc.tensor.matmul(
                        op_[:],
                        v_sb[:, jc * 130 + h * 65:jc * 130 + h * 65 + 65],
                        pts[(h, jc)][:, bass.ts(half, 512)],
                        start=(jc == 0), stop=(jc == 15))

                op_a = avaccp.tile([65, 512], F32, tag="av")
                for jc in range(16):
                    if gate is not None:
                        gate(jc)
                    sgA = accp.tile([128, 1024], F32, tag="acc")
                    sgB = accp.tile([128, 1024], F32, tag="acc")
                    jsl = bass.ts(jc, 128)
                    # row-tiled: h0 rows 0-63 / h1 rows 64-127 run
                    # concurrently in disjoint row-groups.  h0 first (ACT
                    # exp, which runs ahead); a lagging DVE trick stalls only
                    # the h1 members and the h0 matmuls cover the wait.
                    for h, sg in ((0, sgA), (1, sgB)):
                        hsl = slice(h * 64, h * 64 + 64)
                        for ih in range(2):
                            nc.tensor.matmul(sg[:, bass.ts(ih, 512)],
                                             k_rope[hsl, jsl],
                                             q_rope[hsl, isl(ih)],
                                             start=True, stop=True)
                    for h, sg in ((0, sgA), (1, sgB)):
                        if EXP_SCHED[jc][h] == "act":
                            pt = ptilp.tile([128, 1024], BF16, tag="pt")
                            nc.scalar.activation(pt[:], sg[:], ACT_EXP,
                                                 scale=SCALE)
                            pts[(h, jc)] = pt[:]
                        else:
                            pt = ptilp.tile([128, 1024], I16, tag="pt")
                            nc.vector.tensor_scalar(
                                pt[:], sg[:], A16 * SCALE, B16,
                                op0=mybir.AluOpType.mult,
                                op1=mybir.AluOpType.add)
                            pts[(h, jc)] = pt[:].bitcast(BF16)
                    if jc >= 3:
                        emit_av(op_a, 0, jc - 3, 0)
                    drive(cad)
                for jc in (13, 14, 15):
                    emit_av(op_a, 0, jc, 0)
                attn_tail(op_a, ot0, slice(0, 64))
                drive_post(4)
                # h1 first half, then both second halves: pure resident-pt
                # replays with no exp dependency; deferred rope combines and
                # out-projections (og staging) drain here.
                op_b = avaccp.tile([65, 512], F32, tag="av")
                for jc in range(16):
                    emit_av(op_b, 1, jc, 0)
                    if jc % 8 == 5:
                        drive_post(1)
                attn_tail(op_b, ot0, slice(64, 128))
                if tail_hook is not None:
                    tail_hook(0)
                drive_post(2)
                op_c = avaccp.tile([65, 512], F32, tag="av")
                for jc in range(16):
                    emit_av(op_c, 0, jc, 1)
                    if jc % 8 == 5:
                        drive_post(1)
                attn_tail(op_c, ot1, slice(0, 64))
                drive_post(2)
                op_d = avaccp.tile([65, 512], F32, tag="av")
                for jc in range(16):
                    emit_av(op_d, 1, jc, 1)
                    if jc % 8 == 5:
                        drive_post(1)
                attn_tail(op_d, ot1, slice(64, 128))

            # Static startup: head-pair 0's first two blocks (sequential:
            # with the single pacc bank, interleaving two blocks deadlocks
            # on the PE FIFO).  Their rope/v-transpose tails are deferred
            # past block 1's matmuls so DVE work overlaps PE work, then
            # drained (q_rope must be complete before the first scores).
            # Blocks 2-3 interleave into unit (0,0)'s attention,
            # force-drained before jc>=8 reads keys past 1024.
            sdefer = []
            for blk in range(2):
                for _ in proj_block_gen(0, blk, sdefer):
                    pass
            while sdefer:
                for _ in sdefer.pop(0):
                    pass
            pending = []
            # pb0/pb1 interleaved so every unit carries ~2 projection blocks
            # (pb0-only ordering left the pb0 units projection-saturated at
            # ~55us while pb1 units idled at ~33us)
            # pb0 completes at unit 5 so ib0/ib1 out-projections drain in
            # units 6-7's stretches instead of piling onto the final tail
            units = [(0, 0), (1, 0), (0, 1), (1, 1), (0, 2), (0, 3),
                     (1, 2), (1, 3)]
            proj_sched = {0: [(0, 2), (0, 3), (1, 0), (1, 1)],
                          1: [(1, 2), (1, 3)],
                          2: [(2, 0), (2, 1)],
                          3: [(2, 2), (2, 3), (3, 0)],
                          4: [(3, 1), (3, 2), (3, 3)]}
            for p, (pb, hp) in enumerate(units):
                fill = []
                fill_post = []
                fill_rope = []
                gate = None
                gens = []
                for php, blk in proj_sched.get(p, []):
                    prefetch_xt(php, blk)
                    g = proj_block_gen(php, blk, fill_rope)
                    gens.append(g)
                    fill.append(g)
                if p == 0:
                    # unit 0 must fill its own blocks 2-3 mid-loop, drained
                    # before jc 8 reads keys past token 1024
                    def gate(jc, gens=tuple(gens[:2]), fr=fill_rope):
                        if jc == 8:
                            for g in gens:
                                for _ in g:
                                    pass
                            while fr:
                                for _ in fr.pop(0):
                                    pass
                # deferred outprojs (only available near the end) spread
                # over the final units' stretches
                for _ in range(2):
                    if pending:
                        fill_post.append(outproj_gen(pending.pop(0)))

                def drive(n, fill=fill):
                    for _ in range(n):
                        while fill:
                            try:
                                next(fill[0])
                                break
                            except StopIteration:
                                fill.pop(0)

                def drive_post(n, fills=(fill_rope, fill_post, fill)):
                    for _ in range(n):
                        for src in fills:
                            if src:
                                break
                        else:
                            return
                        try:
                            next(src[0])
                        except StopIteration:
                            src.pop(0)

                ot0 = outTp.tile([128, 512], BF16, tag="ot")
                ot1 = outTp.tile([128, 512], BF16, tag="ot")
                ots[(hp, pb, 0)] = ot0
                ots[(hp, pb, 1)] = ot1
                last = p == len(units) - 1

                def hook(half, fill_post=fill_post, pb=pb):
                    # last unit: overlap the first-half out-projection with
                    # the phase-C/D replays instead of the serial drain
                    fill_post.append(outproj_gen(2 * pb))

                attn_unit(hp, pb, ot0, ot1, drive, drive_post,
                          tail_hook=hook if last else None, gate=gate,
                          cad=3 if len(proj_sched.get(p, [])) >= 3 else 2)
                drive_post(99)
                drive(99)
                if hp == NHP - 1:
                    if last:
                        pending.append(2 * pb + 1)
                    else:
                        pending.append(2 * pb)
                        pending.append(2 * pb + 1)
            for ib in pending:
                for _ in outproj_gen(ib):
                    pass

    nc.compile()
    return nc


_NC = None


def _get_nc():
    global _NC
    if _NC is None:
        _NC = _build_program()
    return _NC


def _host_prep(x, w_qkv, w_fproj, b_fproj, w_out, b_out):
    bt = lambda a: np.ascontiguousarray(np.asarray(a, dtype=np.float32),
                                        dtype=np.float32).astype(NP_BF16)
    xTf = bt(x.reshape(B * N, DIM).T)  # [DIM, B*N]

    pos = np.arange(N, dtype=np.float64)[:, None]
    freqs = 10000.0 ** (-np.arange(0, DH, 2, dtype=np.float64) / DH)
    ang = pos * freqs
    sin = np.repeat(np.sin(ang), 2, axis=1)  # [N, 64] interleave-dup
    cos = np.repeat(np.cos(ang), 2, axis=1)
    cos2 = np.tile(cos.T, (2, 1)).astype(NP_BF16)
    sin2 = np.tile(sin.T, (2, 1)).astype(NP_BF16)
    ff = np.arange(1, NF + 1, dtype=np.float64)
    fourier = np.concatenate([np.sin(pos * ff), np.cos(pos * ff)], axis=1)
    fourT = fourier.T.astype(NP_BF16)
    wfT = bt(w_fproj.T)
    bff = np.ascontiguousarray(b_fproj[:, None], dtype=np.float32)
    onesv = np.ones((128, 32), dtype=NP_BF16)

    # rotate_half as a signed permutation: rot(q)[d] = sign[d] * q[perm[d]]
    perm = np.empty(DH, np.int64)
    sign = np.empty(DH, np.float32)
    perm[:32] = 2 * np.arange(32) + 1
    sign[:32] = -1.0
    perm[32:] = 2 * np.arange(32)
    sign[32:] = 1.0
    identm = np.eye(128, dtype=NP_BF16)
    rotmT = np.zeros((128, 128), dtype=NP_BF16)
    for hb in range(2):
        for dl in range(DH):
            rotmT[hb * DH + perm[dl], hb * DH + dl] = sign[dl]

    in_maps = []
    for c in range(NCORES):
        bb = c % B
        hg = c // B  # head group: heads 8*hg .. 8*hg+7
        rows = np.arange(hg * 8 * DH, (hg + 1) * 8 * DH)  # 512 inner dims
        Wq = w_qkv[rows]          # [512, DIM]
        Wk = w_qkv[INNER + rows]
        Wv = w_qkv[2 * INNER + rows]

        in_maps.append({
            "xT": np.ascontiguousarray(xTf[:, bb * N:(bb + 1) * N]),
            "wq": bt(Wq.T), "wk": bt(Wk.T), "wv": bt(Wv.T),
            "rotmT": rotmT,
            "wo": bt(w_out[:, rows].T),  # [512, DIM]
            "cos2": cos2, "sin2": sin2,
            "fourT": fourT, "wfT": wfT, "bf": bff, "ident": identm,
            "onesv": onesv,
        })
    return in_maps


LAST_RESULT = None


def kernel(x, w_qkv, w_fproj, b_fproj, w_out, b_out, *, trace=False):
    global LAST_RESULT
    x = np.asarray(x, dtype=np.float32)
    w_qkv = np.asarray(w_qkv, dtype=np.float32)
    w_fproj = np.asarray(w_fproj, dtype=np.float32)
    b_fproj = np.asarray(b_fproj, dtype=np.float32)
    w_out = np.asarray(w_out, dtype=np.float32)
    b_out = np.asarray(b_out, dtype=np.float32)

    nc = _get_nc()
    in_maps = _host_prep(x, w_qkv, w_fproj, b_fproj, w_out, b_out)
    res = run_bass_kernel_spmd(nc, in_maps, core_ids=list(range(NCORES)),
                               trace=trace)
    LAST_RESULT = res
    acc = np.zeros((B, N, DIM), dtype=np.float64)
    for c in range(NCORES):
        acc[c % B] += res.results[c]["out"].astype(np.float64)
    acc += b_out
    return acc.astype(np.float32)
